# revision 1
# baseline (speedup 1.0000x reference)
"""Causal self-attention (B=1, T=4096, C=768, H=12) on 8 Trainium2 NeuronCores.

Sharding: 24 units = (head, query-half).  Each core owns one full head
(slot 0, all 8 causal-balanced query pairs) plus one half head (slot 1,
4 pairs) -- 1.5 heads of attention area per core, no dummy slots.
Cores 0-3 run a program variant whose slot 1 covers pairs 0-3; cores
4-7 run the complementary variant (pairs 4-7).  The two variants have
identical instruction streams (only column constants differ), so their
simulated/hardware cost is identical.

Head map: core c in 0..3: slot0 = head c, slot1 = head 8+c (pairs 0-3)
          core c in 4..7: slot0 = head c, slot1 = head 4+c (pairs 4-7)

Per core:
  1. x^T arrives pre-transposed and pre-cast to bf16 by the host (a
     sharding-layout choice; no PE transposes anywhere).  Q^T/K^T are
     projected per 512-col t-slice into [128, T] (2 heads x 64 dims on
     partitions); V is projected directly in natural [t, d] layout with
     an appended ones column that accumulates the softmax denominator
     during P@V.  QKV work beyond what the first pair needs streams
     into the attention phase as fillers (the serialized x^T DMA makes
     the start window precious).
  2. Flash-style causal attention, q-blocks in balanced pairs (i, 15-i):
     S^T = K^T.T @ Q^T per 128-wide k-block (bf16), causal masking via
     a -1e30 bias add on the diagonal blocks (DVE), P = exp(S^T/8) on
     the Act engine (the kernel's bottleneck: ~107us of exp at 1
     col/cycle), P@V per k-block in bf16 accumulating y^T + denominator
     in PSUM.  (fp8 DoubleRow P@V was tried and reverted: DoubleRow
     ldweights cap the output at 64 partitions, evicting the
     denominator row, and a separate denominator matmul costs the
     savings back.  GPSIMD cannot touch PSUM, so staging rides DVE.)
  3. y^T normalized via reciprocal + ones-broadcast matmul; partial
     output projection (bank-aligned 512/256 chunks through the shared
     1-bank PSUM pool); partial sums DMA'd out in bf16 and summed on
     the host.  Norm/proj run as deferred closures drained a full pair
     behind the attention front so their dependencies are always ready
     (the PE wait-queue is strict in-order; a stalled matmul blocks
     everything behind it).
"""

import sys

sys.path.insert(0, "/opt/trn_rl_repo")

import numpy as np

T = 4096
C = 768
H = 12
HD = 64
N_CORES = 8
QB = 256  # q-block rows
NQB = T // QB  # 16
KB = 128  # k-block rows
NKB = T // KB  # 32
NPAIR = NQB // 2  # 8 causal-balanced pairs (i, 15-i)
TS = 512  # t-slice for Q/K projection
NTS = T // TS  # 8

_CACHE = {}


def _paired_col(b256: int) -> int:
    """Column offset of 256-row q-block b256 in the paired SBUF layout."""
    p = min(b256, NQB - 1 - b256)
    side = 1 if b256 >= NQB // 2 else 0
    return 2 * QB * p + QB * side


def _build_nc(variant: int):
    import concourse.bacc as bacc
    import concourse.tile as tile
    import concourse.mybir as mybir
    from contextlib import ExitStack
    import collections

    F32 = mybir.dt.float32
    F32R = mybir.dt.float32r
    BF16 = mybir.dt.bfloat16
    FP8 = mybir.dt.float8e4
    EXP = mybir.ActivationFunctionType.Exp
    DR = mybir.MatmulPerfMode.DoubleRow

    s1_pairs = (0, 1, 2, 3) if variant == 0 else (4, 5, 6, 7)

    nc = bacc.Bacc(
        "TRN2",
        target_bir_lowering=False,
        debug=False,
        enable_asserts=True,
        num_devices=N_CORES // 2,
    )
    xt_d = nc.dram_tensor("xt", [C, T], BF16, kind="ExternalInput")
    wa_d = nc.dram_tensor("wa", [C, 3 * 2 * HD], BF16, kind="ExternalInput")
    wp_d = nc.dram_tensor("wp", [2 * HD, C], BF16, kind="ExternalInput")
    out_d = nc.dram_tensor("out", [T, C], BF16, kind="ExternalOutput")

    with ExitStack() as ctx:
        tc = ctx.enter_context(tile.TileContext(nc))
        singles = ctx.enter_context(tc.tile_pool(name="singles", bufs=1))
        ptpool = ctx.enter_context(tc.tile_pool(name="ptpool", bufs=3))
        rpool = ctx.enter_context(tc.tile_pool(name="rpool", bufs=4))
        opool = ctx.enter_context(tc.tile_pool(name="opool", bufs=4))
        ps = ctx.enter_context(tc.tile_pool(name="ps", bufs=2, space="PSUM"))
        ps_st = ctx.enter_context(tc.tile_pool(name="ps_st", bufs=2, space="PSUM"))
        ps_yt = ctx.enter_context(tc.tile_pool(name="ps_yt", bufs=2, space="PSUM"))

        # ---- persistent SBUF tensors ----
        xt6 = singles.tile([128, 6, T], BF16)  # x^T, c-chunk major
        qt = singles.tile([128, T], BF16)  # Q^T, paired column layout
        kt = singles.tile([128, T], BF16)  # K^T, natural column layout
        yt_all = singles.tile([128, T], BF16)  # normalized y^T, paired layout
        # V natural blocks (bf16) + ones col: [k-part, kb, slot, d+1]
        v1 = singles.tile([128, NKB, 2, HD + 1], BF16)
        wa_sb = singles.tile([128, 6, 3 * 2 * HD], BF16)
        wp_sb = singles.tile([2 * HD, C], BF16)
        ones64 = singles.tile([1, HD], F32R)
        # mbias[p, j, f] = 0 where f - p - 128j >= 0 (q >= k), else -1e30
        mbias = singles.tile([128, 2, QB], F32)

        ones_f32 = singles.tile([1, HD], F32)
        nc.gpsimd.memset(ones_f32, 1.0)
        nc.vector.tensor_copy(out=ones64, in_=ones_f32)
        vone_f32 = singles.tile([128, NKB * 2], F32)
        nc.gpsimd.memset(vone_f32, 1.0)
        nc.gpsimd.tensor_copy(
            out=v1[:, :, :, HD : HD + 1],
            in_=vone_f32.rearrange("p (a s) -> p a s", a=NKB).unsqueeze(3),
        )

        # keep -1e30 where f < p + 128j, fill 0.0 where f >= p + 128j
        nc.gpsimd.memset(mbias, -1.0e30)
        for j in range(2):
            nc.gpsimd.affine_select(
                out=mbias[:, j, :],
                in_=mbias[:, j, :],
                compare_op=mybir.AluOpType.is_gt,
                fill=0.0,
                base=128 * j,
                channel_multiplier=1,
                pattern=[[-1, QB]],
            )

        # wa as one strided DMA (HWDGE dispatch is globally serialized at
        # ~625ns/DMA, so fewer DMAs win), then the six x^T chunks spread
        # over the three DGE queues
        nc.sync.dma_start(
            out=wa_sb, in_=wa_d.ap().rearrange("(a p) f -> p a f", p=128)
        )
        # split at t=2560: the upfront K0-4/Q3-4/V0-9 groups only read
        # t < 2560, so the first exp fires ~8us earlier than waiting for
        # whole chunks (DMA transfers serialize in the shared pool)
        xt_q = [nc.sync, nc.scalar, nc.gpsimd]
        for t0, t1 in ((0, 2560), (2560, 4096)):
            for i in range(6):
                xt_q[i % 3].dma_start(
                    out=xt6[:, i, t0:t1],
                    in_=xt_d.ap()[i * 128 : (i + 1) * 128, t0:t1],
                )
        nc.gpsimd.dma_start(out=wp_sb, in_=wp_d.ap())

        # ---- QKV emission units (each: one PSUM group + copy-out) ----
        def emit_q(ts):
            t0 = ts * TS
            pp = ps.tile([128, TS], F32, name="pp", tag="ps")
            for ic in range(6):
                nc.tensor.matmul(
                    pp,
                    lhsT=wa_sb[:, ic, 0:128],
                    rhs=xt6[:, ic, t0 : t0 + TS],
                    start=(ic == 0),
                    stop=(ic == 5),
                )
            for half in range(2):
                col = _paired_col(2 * ts + half)
                nc.vector.tensor_copy(
                    out=qt[:, col : col + QB],
                    in_=pp[:, half * QB : (half + 1) * QB],
                )

        def emit_k(ts):
            t0 = ts * TS
            pp = ps.tile([128, TS], F32, name="pp", tag="ps")
            for ic in range(6):
                nc.tensor.matmul(
                    pp,
                    lhsT=wa_sb[:, ic, 128:256],
                    rhs=xt6[:, ic, t0 : t0 + TS],
                    start=(ic == 0),
                    stop=(ic == 5),
                )
            nc.vector.tensor_copy(out=kt[:, t0 : t0 + TS], in_=pp)

        def emit_v(tb):
            vp = ps.tile([128, 128], F32, name="vp", tag="ps")
            for ic in range(6):
                nc.tensor.matmul(
                    vp,
                    lhsT=xt6[:, ic, tb * 128 : (tb + 1) * 128],
                    rhs=wa_sb[:, ic, 256:384],
                    start=(ic == 0),
                    stop=(ic == 5),
                )
            nc.vector.tensor_copy(
                out=v1[:, tb, :, 0:HD],
                in_=vp.rearrange("p (s d) -> p s d", s=2),
            )

        # ---- deferred-work streams ----
        scale = 1.0 / float(np.sqrt(HD))
        work_q = collections.deque()  # norm/proj closures (lag >= 1 pair)
        fillers = collections.deque()  # remaining QKV units

        # Deferred norm/proj work runs as a 2-stage pipeline: stage 1 emits
        # the PE matmuls, stage 2 (enqueued when stage 1 drains, so it pops
        # at a later group) emits their DVE consumers.  That way DVE-queue
        # entries are nearly-ready when enqueued and never head-of-line
        # block the prompt-class mask adds / staging copies.  Nothing
        # drains in a pair's first two groups (protects the Act engine's
        # restart cadence at pair boundaries).
        def drain_one(g):
            # throttle injections: one deferred item per group keeps the
            # exp cadence close to back-to-back; groups that already took
            # a QKV filler skip work items unless the queue backs up
            took_filler = False
            if fillers:
                fillers.popleft()()
                took_filler = True
            if g < 2:
                return
            if took_filler and len(work_q) <= 8:
                return
            n = 2 if len(work_q) > 8 else min(1, len(work_q))
            for _ in range(min(n, len(work_q))):
                work_q.popleft()()

        def emit_proj(tb, full):
            def s1():
                b256, half = tb // 2, tb % 2
                col = _paired_col(b256) + 128 * half
                r1 = 128 if full else HD
                pos = []
                for c0, c1 in ((0, 512), (512, 768)):
                    po = ps.tile([128, c1 - c0], F32, name="po", tag="ps")
                    nc.tensor.matmul(
                        po,
                        lhsT=yt_all[0:r1, col : col + 128],
                        rhs=wp_sb[0:r1, c0:c1],
                        start=True,
                        stop=True,
                    )
                    pos.append(po)

                def s2():
                    osb = opool.tile([128, C], BF16, name="osb", tag="osb")
                    for (c0, c1), po in zip(((0, 512), (512, 768)), pos):
                        nc.vector.tensor_copy(out=osb[:, c0:c1], in_=po)
                    nc.sync.dma_start(
                        out=out_d.ap()[tb * 128 : (tb + 1) * 128, :], in_=osb
                    )

                work_q.append(s2)

            return s1

        def emit_norm(ytsb, r0, r1, col, after=()):
            """after: closures enqueued once this norm's mul has been
            emitted (used to order proj emission behind yt_all writes)."""

            def s1():
                r_sb = rpool.tile([1, 2 * QB], F32R, name="r_sb", tag="r_sb", bufs=8)
                with nc.allow_low_precision(reason="softmax denom broadcast"):
                    nc.vector.reciprocal(out=r_sb, in_=ytsb[HD : HD + 1, :])
                bc = ps.tile([HD, 2 * QB], F32, name="bc", tag="ps")
                nc.tensor.matmul(bc, lhsT=ones64, rhs=r_sb, start=True, stop=True)

                def s2():
                    nc.vector.tensor_mul(
                        out=yt_all[r0:r1, col : col + 2 * QB],
                        in0=ytsb[0:HD, :],
                        in1=bc,
                    )
                    work_q.extend(after)

                work_q.append(s2)

            return s1

        def emit_pair(s, i, after=()):
            """Attention for slot s (d-rows [64s, 64s+64)), pair i."""
            r0, r1 = s * HD, (s + 1) * HD
            qcol = 2 * QB * i
            n_shared = 2 * i + 2
            n_total = NKB - 2 * i
            diag_b0 = NKB - 2 - 2 * i  # first diagonal k-block of side B
            yt = ps_yt.tile([HD + 1, 2 * QB], F32, name="yt", tag="yt")
            groups = [(g, g + 2) for g in range(0, n_shared, 2)]
            kb0 = n_shared
            while kb0 < n_total:
                n = min(4, n_total - kb0)
                groups.append((kb0, kb0 + n))
                kb0 += n

            def emit_s(grp):
                ka, kb = grp
                shared = ka < n_shared
                w = 2 * QB if shared else QB
                qoff = qcol if shared else qcol + QB
                gw = w * (kb - ka)
                st = ps_st.tile([128, 4 * QB], F32, name="st", tag="st")
                for j in range(kb - ka):
                    blk = ka + j
                    nc.tensor.matmul(
                        st[:, j * w : (j + 1) * w],
                        lhsT=kt[r0:r1, blk * KB : (blk + 1) * KB],
                        rhs=qt[r0:r1, qoff : qoff + w],
                        start=True,
                        stop=True,
                    )
                # causal mask: -1e30 bias onto the diagonal blocks' own half
                for j in range(kb - ka):
                    blk = ka + j
                    if shared and blk in (2 * i, 2 * i + 1):
                        nc.vector.tensor_add(
                            out=st[:, j * w : j * w + QB],
                            in0=st[:, j * w : j * w + QB],
                            in1=mbias[:, blk - 2 * i, :],
                        )
                    elif not shared and blk in (diag_b0, diag_b0 + 1):
                        nc.vector.tensor_add(
                            out=st[:, j * w : (j + 1) * w],
                            in0=st[:, j * w : (j + 1) * w],
                            in1=mbias[:, blk - diag_b0, :],
                        )
                pt = ptpool.tile([128, 4 * QB], BF16, name="pt", tag="pt")
                nc.scalar.activation(
                    out=pt[:, 0:gw], in_=st[:, 0:gw], func=EXP, scale=scale
                )
                return pt, w

            def emit_pv(grp, pt, w):
                ka, kb = grp
                shared = ka < n_shared
                for j in range(kb - ka):
                    blk = ka + j
                    if shared:
                        nc.tensor.matmul(
                            yt,
                            lhsT=v1[:, blk, s, :],
                            rhs=pt[:, j * w : (j + 1) * w],
                            start=(blk == 0),
                            stop=False,
                            skip_group_check=True,
                        )
                    else:
                        nc.tensor.matmul(
                            yt[:, QB : 2 * QB],
                            lhsT=v1[:, blk, s, :],
                            rhs=pt[:, j * w : (j + 1) * w],
                            start=False,
                            stop=(blk == n_total - 1),
                            skip_group_check=True,
                        )

            pending = None
            for gi, grp in enumerate(groups):
                cur = (grp, *emit_s(grp))
                if pending is not None:
                    emit_pv(*pending)
                pending = cur
                drain_one(gi)
            emit_pv(*pending)
            # free the yt PSUM slot quickly; normalization is deferred
            ytsb = rpool.tile([HD + 1, 2 * QB], F32, name="ytsb", tag="ytsb", bufs=6)
            nc.vector.tensor_copy(out=ytsb, in_=yt)
            work_q.append(emit_norm(ytsb, r0, r1, qcol, after))

        # ---- schedule ----
        # Pair processing order: causal-need ascending at the front (pair 7
        # needs the least K/V), and a slot0-only pair LAST so the final
        # norm->proj tail is short and runs on a warm PE.
        order = [7, 6, 5, 3, 2, 1, 0, 4]

        # upfront: exactly what pair 7 needs at its start
        for ts in range(5):
            emit_k(ts)
        emit_q(3)
        emit_q(4)
        for tb in range(10):
            emit_v(tb)

        # remaining QKV units with due POSITION in the processing order
        # (first position whose pair consumes them; V for a pair's own
        # tail k-blocks gets one position of stream-in slack)
        due = {
            1: [lambda tb=tb: emit_v(tb) for tb in range(10, 18)],
            2: [lambda: emit_q(2), lambda: emit_q(5)]
            + [lambda tb=tb: emit_v(tb) for tb in range(18, 20)],
            3: [lambda: emit_k(5), lambda: emit_k(6), lambda: emit_q(1),
                lambda: emit_q(6)]
            + [lambda tb=tb: emit_v(tb) for tb in range(20, 24)],
            4: [lambda tb=tb: emit_v(tb) for tb in range(24, 26)],
            5: [lambda: emit_k(7), lambda: emit_q(0), lambda: emit_q(7)]
            + [lambda tb=tb: emit_v(tb) for tb in range(26, 28)],
            6: [lambda tb=tb: emit_v(tb) for tb in range(28, 30)],
            7: [lambda tb=tb: emit_v(tb) for tb in range(30, 32)],
        }
        for p in range(1, NPAIR):
            fillers.extend(due.get(p, []))
        # fillers allowed to remain when position p starts = units due later
        allowed = {
            p: sum(len(due.get(j, [])) for j in range(p + 1, NPAIR))
            for p in range(NPAIR)
        }

        for p, i in enumerate(order):
            while len(fillers) > allowed[p]:
                fillers.popleft()()
            full = i in s1_pairs
            projs = [
                emit_proj(tb, full)
                for tb in (2 * i, 2 * i + 1, NKB - 2 - 2 * i, NKB - 1 - 2 * i)
            ]
            emit_pair(0, i, after=() if full else projs)
            if full:
                emit_pair(1, i, after=projs)
        while fillers:
            fillers.popleft()()
        while work_q:
            work_q.popleft()()

    nc.compile()
    return nc


def _get_ncs():
    if "ncs" not in _CACHE:
        _CACHE["ncs"] = [_build_nc(0), _build_nc(1)]
    return _CACHE["ncs"]


def _core_inputs(x, w_attn, w_proj):
    """Build per-core input dicts (bf16, pre-transposed x, head slices)."""
    import ml_dtypes

    bf16 = ml_dtypes.bfloat16
    xt = np.ascontiguousarray(x.reshape(T, C).T.astype(bf16))
    w_attn = np.asarray(w_attn, dtype=np.float32)
    w_proj = np.asarray(w_proj, dtype=np.float32)
    in_maps = []
    for c in range(N_CORES):
        hF = c
        hH = 8 + (c % 4)
        wa = np.zeros((C, 3, 2, HD), dtype=np.float32)
        wp = np.zeros((2 * HD, C), dtype=np.float32)
        for s, h in enumerate((hF, hH)):
            for p in range(3):
                wa[:, p, s, :] = w_attn[:, p * C + h * HD : p * C + (h + 1) * HD]
            wp[s * HD : (s + 1) * HD, :] = w_proj[h * HD : (h + 1) * HD, :]
        in_maps.append(
            {
                "xt": xt,
                "wa": np.ascontiguousarray(wa.reshape(C, 3 * 2 * HD)).astype(bf16),
                "wp": wp.astype(bf16),
            }
        )
    return in_maps


def _make_sharded(nc, devices):
    """Build one 4-core shard_map'd PJRT executable for a program variant."""
    import jax
    import concourse.mybir as mybir
    from concourse import bass2jax
    from jax.experimental.shard_map import shard_map
    from jax.sharding import Mesh, PartitionSpec

    in_names, out_names, out_avals, zero_outs = [], [], [], []
    for alloc in nc.m.functions[0].allocations:
        if not isinstance(alloc, mybir.MemoryLocationSet):
            continue
        name = alloc.memorylocations[0].name
        if alloc.kind == "ExternalInput":
            if nc.partition_id_tensor and name == nc.partition_id_tensor.name:
                continue
            in_names.append(name)
        elif alloc.kind == "ExternalOutput":
            shape = tuple(alloc.tensor_shape)
            dtype = mybir.dt.np(alloc.dtype)
            out_names.append(name)
            out_avals.append(jax.core.ShapedArray(shape, dtype))
            zero_outs.append(np.zeros(shape, dtype))
    n_params = len(in_names)
    all_in_names = in_names + out_names
    if nc.partition_id_tensor:
        all_in_names = all_in_names + [nc.partition_id_tensor.name]

    def _body(*args):
        operands = list(args)
        if nc.partition_id_tensor:
            operands.append(bass2jax.partition_id_tensor())
        outs = bass2jax._bass_exec_p.bind(
            *operands,
            out_avals=tuple(out_avals),
            in_names=tuple(all_in_names),
            out_names=tuple(out_names),
            lowering_input_output_aliases=(),
            sim_require_finite=True,
            sim_require_nnan=True,
            nc=nc,
        )
        return tuple(outs)

    mesh = Mesh(np.asarray(devices), ("core",))
    n_out = len(out_names)
    donate = tuple(range(n_params, n_params + n_out))
    sharded = jax.jit(
        shard_map(
            _body,
            mesh=mesh,
            in_specs=(PartitionSpec("core"),) * (n_params + n_out),
            out_specs=(PartitionSpec("core"),) * n_out,
            check_rep=False,
        ),
        donate_argnums=donate,
        keep_unused=True,
    )
    return sharded, in_names, out_names, out_avals, zero_outs


def _get_runner():
    if "runner" in _CACHE:
        return _CACHE["runner"]
    import jax
    from concourse import bass2jax

    ncs = _get_ncs()
    bass2jax.install_neuronx_cc_hook()
    devices = jax.devices()[:N_CORES]
    execs = [
        _make_sharded(ncs[0], devices[0:4]),
        _make_sharded(ncs[1], devices[4:8]),
    ]

    def run(in_maps):
        results = [None] * N_CORES
        pending = []
        for v, (sharded, in_names, out_names, out_avals, zero_outs) in enumerate(
            execs
        ):
            cores = range(4 * v, 4 * v + 4)
            concat_in = [
                np.concatenate([in_maps[c][name] for c in cores], axis=0)
                for name in in_names
            ]
            concat_zeros = [
                np.zeros((4 * z.shape[0], *z.shape[1:]), z.dtype) for z in zero_outs
            ]
            out_arrs = sharded(*concat_in, *concat_zeros)
            pending.append((v, out_names, out_avals, out_arrs))
        for v, out_names, out_avals, out_arrs in pending:
            for i, name in enumerate(out_names):
                arr = np.asarray(out_arrs[i]).reshape(4, *out_avals[i].shape)
                for j in range(4):
                    c = 4 * v + j
                    if results[c] is None:
                        results[c] = {}
                    results[c][name] = arr[j]
        return results

    _CACHE["runner"] = run
    return run


def kernel(x, w_attn, w_proj):
    run = _get_runner()
    in_maps = _core_inputs(np.asarray(x), np.asarray(w_attn), np.asarray(w_proj))
    results = run(in_maps)
    out = np.zeros((T, C), dtype=np.float32)
    for c in range(N_CORES):
        out += results[c]["out"].astype(np.float32)
    return out.reshape(1, T, C)



# revision 7
# speedup vs baseline: 1.1257x; 1.1257x over previous
"""Causal self-attention (B=1, T=4096, C=768, H=12) on 8 Trainium2 NeuronCores.

Sharding: 24 units = (head, query-half).  Each core owns one full head
(slot 0, all 8 causal-balanced query pairs) plus one half head (slot 1,
4 pairs) -- 1.5 heads of attention area per core, no dummy slots.
Cores 0-3 run a program variant whose slot 1 covers pairs 0-3; cores
4-7 run the complementary variant (pairs 4-7).  The two variants have
identical instruction streams (only column constants differ), so their
simulated/hardware cost is identical.

Head map: core c in 0..3: slot0 = head c, slot1 = head 8+c (pairs 0-3)
          core c in 4..7: slot0 = head c, slot1 = head 4+c (pairs 4-7)

Per core:
  1. x^T arrives pre-transposed and pre-cast to bf16 by the host (a
     sharding-layout choice; no PE transposes anywhere).  Q^T/K^T are
     projected per 512-col t-slice into [128, T] (2 heads x 64 dims on
     partitions); V is projected directly in natural [t, d] layout with
     an appended ones column that accumulates the softmax denominator
     during P@V.  QKV work beyond what the first pair needs streams
     into the attention phase as fillers (the serialized x^T DMA makes
     the start window precious).
  2. Flash-style causal attention, q-blocks in balanced pairs (i, 15-i):
     S^T = K^T.T @ Q^T per 128-wide k-block (bf16), causal masking via
     a -1e30 bias add on the diagonal blocks (DVE), P = exp(S^T/8) on
     the Act engine (the kernel's bottleneck: ~107us of exp at 1
     col/cycle), P@V per k-block in bf16 accumulating y^T + denominator
     in PSUM.  (fp8 DoubleRow P@V was tried and reverted: DoubleRow
     ldweights cap the output at 64 partitions, evicting the
     denominator row, and a separate denominator matmul costs the
     savings back.  GPSIMD cannot touch PSUM, so staging rides DVE.)
  3. y^T normalized via reciprocal + ones-broadcast matmul; partial
     output projection (bank-aligned 512/256 chunks through the shared
     1-bank PSUM pool); partial sums DMA'd out in bf16 and summed on
     the host.  Norm/proj run as deferred closures drained a full pair
     behind the attention front so their dependencies are always ready
     (the PE wait-queue is strict in-order; a stalled matmul blocks
     everything behind it).
"""

import sys

sys.path.insert(0, "/opt/trn_rl_repo")

import numpy as np

T = 4096
C = 768
H = 12
HD = 64
N_CORES = 8
QB = 256  # q-block rows
NQB = T // QB  # 16
KB = 128  # k-block rows
NKB = T // KB  # 32
NPAIR = NQB // 2  # 8 causal-balanced pairs (i, 15-i)
TS = 512  # t-slice for Q/K projection
NTS = T // TS  # 8

_CACHE = {}


def _paired_col(b256: int) -> int:
    """Column offset of 256-row q-block b256 in the paired SBUF layout."""
    p = min(b256, NQB - 1 - b256)
    side = 1 if b256 >= NQB // 2 else 0
    return 2 * QB * p + QB * side


def _build_nc(variant: int):
    import concourse.bacc as bacc
    import concourse.tile as tile
    import concourse.mybir as mybir
    from contextlib import ExitStack
    import collections

    F32 = mybir.dt.float32
    F32R = mybir.dt.float32r
    BF16 = mybir.dt.bfloat16
    FP8 = mybir.dt.float8e4
    EXP = mybir.ActivationFunctionType.Exp
    DR = mybir.MatmulPerfMode.DoubleRow

    s1_pairs = (0, 1, 2, 3) if variant == 0 else (4, 5, 6, 7)

    nc = bacc.Bacc(
        "TRN2",
        target_bir_lowering=False,
        debug=False,
        enable_asserts=True,
        num_devices=N_CORES // 2,
    )
    xt_d = nc.dram_tensor("xt", [C, T], BF16, kind="ExternalInput")
    wa_d = nc.dram_tensor("wa", [C, 3 * 2 * HD], BF16, kind="ExternalInput")
    wp_d = nc.dram_tensor("wp", [2 * HD, C], BF16, kind="ExternalInput")
    out_d = nc.dram_tensor("out", [T, C], BF16, kind="ExternalOutput")

    with ExitStack() as ctx:
        tc = ctx.enter_context(tile.TileContext(nc))
        singles = ctx.enter_context(tc.tile_pool(name="singles", bufs=1))
        ptpool = ctx.enter_context(tc.tile_pool(name="ptpool", bufs=4))
        rpool = ctx.enter_context(tc.tile_pool(name="rpool", bufs=4))
        opool = ctx.enter_context(tc.tile_pool(name="opool", bufs=4))
        ps = ctx.enter_context(tc.tile_pool(name="ps", bufs=2, space="PSUM"))
        ps_st = ctx.enter_context(tc.tile_pool(name="ps_st", bufs=2, space="PSUM"))
        ps_yt = ctx.enter_context(tc.tile_pool(name="ps_yt", bufs=2, space="PSUM"))

        # ---- persistent SBUF tensors ----
        xt6 = singles.tile([128, 6, T], BF16)  # x^T, c-chunk major
        qt = singles.tile([128, T], BF16)  # Q^T, paired column layout
        kt = singles.tile([128, T], BF16)  # K^T, natural column layout
        yt_all = singles.tile([128, T], BF16)  # normalized y^T, paired layout
        # V natural blocks (bf16) + ones col: [k-part, kb, slot, d+1]
        v1 = singles.tile([128, NKB, 2, HD + 1], BF16)
        wa_sb = singles.tile([128, 6, 3 * 2 * HD], BF16)
        wp_sb = singles.tile([2 * HD, C], BF16)
        ones64 = singles.tile([1, HD], F32R)

        ones_f32 = singles.tile([1, HD], F32)
        nc.gpsimd.memset(ones_f32, 1.0)
        nc.vector.tensor_copy(out=ones64, in_=ones_f32)
        vone_f32 = singles.tile([128, NKB * 2], F32)
        nc.gpsimd.memset(vone_f32, 1.0)
        nc.gpsimd.tensor_copy(
            out=v1[:, :, :, HD : HD + 1],
            in_=vone_f32.rearrange("p (a s) -> p a s", a=NKB).unsqueeze(3),
        )

        # cached fill register for the post-exp causal-mask zeroing on gpsimd
        zfill = nc.gpsimd.to_reg(0.0)

        # wa as one strided DMA (HWDGE dispatch is globally serialized at
        # ~625ns/DMA, so fewer DMAs win), then x^T in first-need-ordered
        # windows: [1792,2304) feeds q-blocks 7,8 (the first S^T), [0,512)
        # feeds K0/V0-3, then K1/K2 windows, then the tails.  DMA transfers
        # serialize in the shared pool, so window order == availability
        # order.  HWDGE DMAs ride the sync (SP) queue to keep the Act
        # queue free for the exp stream; every third window chunk goes
        # SWDGE via gpsimd.
        nc.sync.dma_start(
            out=wa_sb, in_=wa_d.ap().rearrange("(a p) f -> p a f", p=128)
        )
        xt_q = [nc.sync, nc.sync, nc.gpsimd]
        qi = 0
        for t0, t1 in (
            (1792, 2304),
            (0, 512),
            (512, 1024),
            (1024, 1536),
            (1536, 1792),
            (2304, 2560),
            (2560, 4096),
        ):
            for i in range(6):
                xt_q[qi % 3].dma_start(
                    out=xt6[:, i, t0:t1],
                    in_=xt_d.ap()[i * 128 : (i + 1) * 128, t0:t1],
                )
                qi += 1
        nc.gpsimd.dma_start(out=wp_sb, in_=wp_d.ap())

        # ---- QKV emission units (each: one PSUM group + copy-out) ----
        def emit_q(ts):
            t0 = ts * TS
            pp = ps.tile([128, TS], F32, name="pp", tag="ps")
            for ic in range(6):
                nc.tensor.matmul(
                    pp,
                    lhsT=wa_sb[:, ic, 0:128],
                    rhs=xt6[:, ic, t0 : t0 + TS],
                    start=(ic == 0),
                    stop=(ic == 5),
                )
            for half in range(2):
                col = _paired_col(2 * ts + half)
                nc.vector.tensor_copy(
                    out=qt[:, col : col + QB],
                    in_=pp[:, half * QB : (half + 1) * QB],
                )

        def emit_k(ts):
            t0 = ts * TS
            pp = ps.tile([128, TS], F32, name="pp", tag="ps")
            for ic in range(6):
                nc.tensor.matmul(
                    pp,
                    lhsT=wa_sb[:, ic, 128:256],
                    rhs=xt6[:, ic, t0 : t0 + TS],
                    start=(ic == 0),
                    stop=(ic == 5),
                )
            nc.vector.tensor_copy(out=kt[:, t0 : t0 + TS], in_=pp)

        def emit_v(tb):
            vp = ps.tile([128, 128], F32, name="vp", tag="ps")
            for ic in range(6):
                nc.tensor.matmul(
                    vp,
                    lhsT=xt6[:, ic, tb * 128 : (tb + 1) * 128],
                    rhs=wa_sb[:, ic, 256:384],
                    start=(ic == 0),
                    stop=(ic == 5),
                )
            nc.vector.tensor_copy(
                out=v1[:, tb, :, 0:HD],
                in_=vp.rearrange("p (s d) -> p s d", s=2),
            )

        # ---- deferred-work streams ----
        scale = 1.0 / float(np.sqrt(HD))
        work_q = collections.deque()  # norm/proj closures (lag >= 1 pair)
        fillers = collections.deque()  # remaining QKV units

        # Deferred norm/proj work runs as a 2-stage pipeline: stage 1 emits
        # the PE matmuls, stage 2 (enqueued when stage 1 drains, so it pops
        # at a later group) emits their DVE consumers.  That way DVE-queue
        # entries are nearly-ready when enqueued and never head-of-line
        # block the prompt-class mask adds / staging copies.  Nothing
        # drains in a pair's first two groups (protects the Act engine's
        # restart cadence at pair boundaries).
        def drain_one(g):
            # throttle injections: one deferred item per group keeps the
            # exp cadence close to back-to-back; groups that already took
            # a QKV filler skip work items unless the queue backs up
            took_filler = False
            if fillers:
                fillers.popleft()()
                took_filler = True
            if g < 2:
                return
            if took_filler and len(work_q) <= 8:
                return
            n = 2 if len(work_q) > 8 else min(1, len(work_q))
            for _ in range(min(n, len(work_q))):
                work_q.popleft()()

        def emit_proj(tb, full):
            def s1():
                b256, half = tb // 2, tb % 2
                col = _paired_col(b256) + 128 * half
                r1 = 128 if full else HD
                pos = []
                for c0, c1 in ((0, 512), (512, 768)):
                    po = ps.tile([128, c1 - c0], F32, name="po", tag="ps")
                    nc.tensor.matmul(
                        po,
                        lhsT=yt_all[0:r1, col : col + 128],
                        rhs=wp_sb[0:r1, c0:c1],
                        start=True,
                        stop=True,
                    )
                    pos.append(po)

                def s2():
                    osb = opool.tile([128, C], BF16, name="osb", tag="osb")
                    for (c0, c1), po in zip(((0, 512), (512, 768)), pos):
                        nc.vector.tensor_copy(out=osb[:, c0:c1], in_=po)
                    nc.sync.dma_start(
                        out=out_d.ap()[tb * 128 : (tb + 1) * 128, :], in_=osb
                    )

                work_q.append(s2)

            return s1

        def emit_norm(ytsb, r0, r1, col, after=()):
            """after: closures enqueued once this norm's mul has been
            emitted (used to order proj emission behind yt_all writes)."""

            def s1():
                r_sb = rpool.tile([1, 2 * QB], F32R, name="r_sb", tag="r_sb", bufs=8)
                with nc.allow_low_precision(reason="softmax denom broadcast"):
                    nc.vector.reciprocal(out=r_sb, in_=ytsb[HD : HD + 1, :])
                bc = ps.tile([HD, 2 * QB], F32, name="bc", tag="ps")
                nc.tensor.matmul(bc, lhsT=ones64, rhs=r_sb, start=True, stop=True)

                def s2():
                    nc.vector.tensor_mul(
                        out=yt_all[r0:r1, col : col + 2 * QB],
                        in0=ytsb[0:HD, :],
                        in1=bc,
                    )
                    work_q.extend(after)

                work_q.append(s2)

            return s1

        def emit_pair(s, i, after=()):
            """Attention for slot s (d-rows [64s, 64s+64)), pair i."""
            r0, r1 = s * HD, (s + 1) * HD
            qcol = 2 * QB * i
            n_shared = 2 * i + 2
            n_total = NKB - 2 * i
            diag_b0 = NKB - 2 - 2 * i  # first diagonal k-block of side B
            yt = ps_yt.tile([HD + 1, 2 * QB], F32, name="yt", tag="yt")
            groups = [(g, g + 2) for g in range(0, n_shared, 2)]
            kb0 = n_shared
            while kb0 < n_total:
                n = min(4, n_total - kb0)
                groups.append((kb0, kb0 + n))
                kb0 += n

            def emit_s(grp):
                ka, kb = grp
                shared = ka < n_shared
                w = 2 * QB if shared else QB
                qoff = qcol if shared else qcol + QB
                gw = w * (kb - ka)
                st = ps_st.tile([128, 4 * QB], F32, name="st", tag="st")
                for j in range(kb - ka):
                    blk = ka + j
                    nc.tensor.matmul(
                        st[:, j * w : (j + 1) * w],
                        lhsT=kt[r0:r1, blk * KB : (blk + 1) * KB],
                        rhs=qt[r0:r1, qoff : qoff + w],
                        start=True,
                        stop=True,
                    )
                pt = ptpool.tile([128, 4 * QB], BF16, name="pt", tag="pt")
                nc.scalar.activation(
                    out=pt[:, 0:gw], in_=st[:, 0:gw], func=EXP, scale=scale
                )
                # causal mask: zero the masked half of diagonal blocks on the
                # (otherwise idle) gpsimd engine, keeping DVE off the
                # S^T -> exp critical path.  keep where f >= p + 128*jj.
                for j in range(kb - ka):
                    blk = ka + j
                    if shared and blk in (2 * i, 2 * i + 1):
                        jj = blk - 2 * i
                        reg = pt[:, j * w : j * w + QB]
                    elif not shared and blk in (diag_b0, diag_b0 + 1):
                        jj = blk - diag_b0
                        reg = pt[:, j * w : (j + 1) * w]
                    else:
                        continue
                    nc.gpsimd.affine_select(
                        out=reg,
                        in_=reg,
                        compare_op=mybir.AluOpType.is_gt,
                        fill=zfill,
                        base=1 - 128 * jj,
                        channel_multiplier=-1,
                        pattern=[[1, QB]],
                    )
                return pt, w

            def emit_pv(grp, pt, w):
                ka, kb = grp
                shared = ka < n_shared
                for j in range(kb - ka):
                    blk = ka + j
                    if shared:
                        nc.tensor.matmul(
                            yt,
                            lhsT=v1[:, blk, s, :],
                            rhs=pt[:, j * w : (j + 1) * w],
                            start=(blk == 0),
                            stop=False,
                            skip_group_check=True,
                        )
                    else:
                        nc.tensor.matmul(
                            yt[:, QB : 2 * QB],
                            lhsT=v1[:, blk, s, :],
                            rhs=pt[:, j * w : (j + 1) * w],
                            start=False,
                            stop=(blk == n_total - 1),
                            skip_group_check=True,
                        )

            # S^T runs two groups ahead of P@V so the Act engine's exp chain
            # stays back-to-back: S(g+1) lands on the PE before exp(g) ends.
            pending = collections.deque()
            for gi, grp in enumerate(groups):
                pending.append((grp, *emit_s(grp)))
                if len(pending) > 2:
                    emit_pv(*pending.popleft())
                drain_one(gi)
            while pending:
                emit_pv(*pending.popleft())
            # free the yt PSUM slot quickly; normalization is deferred
            ytsb = rpool.tile([HD + 1, 2 * QB], F32, name="ytsb", tag="ytsb", bufs=6)
            nc.vector.tensor_copy(out=ytsb, in_=yt)
            work_q.append(emit_norm(ytsb, r0, r1, qcol, after))

        # ---- schedule ----
        # Pair processing order: causal-need ascending at the front (pair 7
        # needs the least K/V), and a slot0-only pair LAST so the final
        # norm->proj tail is short and runs on a warm PE.
        order = [7, 6, 5, 3, 2, 1, 0, 4]

        # upfront: exactly what pair 7 needs at its start
        for ts in range(5):
            emit_k(ts)
        emit_q(3)
        emit_q(4)
        for tb in range(10):
            emit_v(tb)

        # remaining QKV units with due POSITION in the processing order
        # (first position whose pair consumes them; V for a pair's own
        # tail k-blocks gets one position of stream-in slack)
        due = {
            1: [lambda tb=tb: emit_v(tb) for tb in range(10, 18)],
            2: [lambda: emit_q(2), lambda: emit_q(5)]
            + [lambda tb=tb: emit_v(tb) for tb in range(18, 20)],
            3: [lambda: emit_k(5), lambda: emit_k(6), lambda: emit_q(1),
                lambda: emit_q(6)]
            + [lambda tb=tb: emit_v(tb) for tb in range(20, 24)],
            4: [lambda tb=tb: emit_v(tb) for tb in range(24, 26)],
            5: [lambda: emit_k(7), lambda: emit_q(0), lambda: emit_q(7)]
            + [lambda tb=tb: emit_v(tb) for tb in range(26, 28)],
            6: [lambda tb=tb: emit_v(tb) for tb in range(28, 30)],
            7: [lambda tb=tb: emit_v(tb) for tb in range(30, 32)],
        }
        for p in range(1, NPAIR):
            fillers.extend(due.get(p, []))
        # fillers allowed to remain when position p starts = units due later
        allowed = {
            p: sum(len(due.get(j, [])) for j in range(p + 1, NPAIR))
            for p in range(NPAIR)
        }

        for p, i in enumerate(order):
            while len(fillers) > allowed[p]:
                fillers.popleft()()
            full = i in s1_pairs
            projs = [
                emit_proj(tb, full)
                for tb in (2 * i, 2 * i + 1, NKB - 2 - 2 * i, NKB - 1 - 2 * i)
            ]
            emit_pair(0, i, after=() if full else projs)
            if full:
                emit_pair(1, i, after=projs)
        while fillers:
            fillers.popleft()()
        while work_q:
            work_q.popleft()()

    nc.compile()
    return nc


def _get_ncs():
    if "ncs" not in _CACHE:
        _CACHE["ncs"] = [_build_nc(0), _build_nc(1)]
    return _CACHE["ncs"]


def _core_inputs(x, w_attn, w_proj):
    """Build per-core input dicts (bf16, pre-transposed x, head slices)."""
    import ml_dtypes

    bf16 = ml_dtypes.bfloat16
    xt = np.ascontiguousarray(x.reshape(T, C).T.astype(bf16))
    w_attn = np.asarray(w_attn, dtype=np.float32)
    w_proj = np.asarray(w_proj, dtype=np.float32)
    in_maps = []
    for c in range(N_CORES):
        hF = c
        hH = 8 + (c % 4)
        wa = np.zeros((C, 3, 2, HD), dtype=np.float32)
        wp = np.zeros((2 * HD, C), dtype=np.float32)
        for s, h in enumerate((hF, hH)):
            for p in range(3):
                wa[:, p, s, :] = w_attn[:, p * C + h * HD : p * C + (h + 1) * HD]
            wp[s * HD : (s + 1) * HD, :] = w_proj[h * HD : (h + 1) * HD, :]
        in_maps.append(
            {
                "xt": xt,
                "wa": np.ascontiguousarray(wa.reshape(C, 3 * 2 * HD)).astype(bf16),
                "wp": wp.astype(bf16),
            }
        )
    return in_maps


def _make_sharded(nc, devices):
    """Build one 4-core shard_map'd PJRT executable for a program variant."""
    import jax
    import concourse.mybir as mybir
    from concourse import bass2jax
    from jax.experimental.shard_map import shard_map
    from jax.sharding import Mesh, PartitionSpec

    in_names, out_names, out_avals, zero_outs = [], [], [], []
    for alloc in nc.m.functions[0].allocations:
        if not isinstance(alloc, mybir.MemoryLocationSet):
            continue
        name = alloc.memorylocations[0].name
        if alloc.kind == "ExternalInput":
            if nc.partition_id_tensor and name == nc.partition_id_tensor.name:
                continue
            in_names.append(name)
        elif alloc.kind == "ExternalOutput":
            shape = tuple(alloc.tensor_shape)
            dtype = mybir.dt.np(alloc.dtype)
            out_names.append(name)
            out_avals.append(jax.core.ShapedArray(shape, dtype))
            zero_outs.append(np.zeros(shape, dtype))
    n_params = len(in_names)
    all_in_names = in_names + out_names
    if nc.partition_id_tensor:
        all_in_names = all_in_names + [nc.partition_id_tensor.name]

    def _body(*args):
        operands = list(args)
        if nc.partition_id_tensor:
            operands.append(bass2jax.partition_id_tensor())
        outs = bass2jax._bass_exec_p.bind(
            *operands,
            out_avals=tuple(out_avals),
            in_names=tuple(all_in_names),
            out_names=tuple(out_names),
            lowering_input_output_aliases=(),
            sim_require_finite=True,
            sim_require_nnan=True,
            nc=nc,
        )
        return tuple(outs)

    mesh = Mesh(np.asarray(devices), ("core",))
    n_out = len(out_names)
    donate = tuple(range(n_params, n_params + n_out))
    sharded = jax.jit(
        shard_map(
            _body,
            mesh=mesh,
            in_specs=(PartitionSpec("core"),) * (n_params + n_out),
            out_specs=(PartitionSpec("core"),) * n_out,
            check_rep=False,
        ),
        donate_argnums=donate,
        keep_unused=True,
    )
    return sharded, in_names, out_names, out_avals, zero_outs


def _get_runner():
    if "runner" in _CACHE:
        return _CACHE["runner"]
    import jax
    from concourse import bass2jax

    ncs = _get_ncs()
    bass2jax.install_neuronx_cc_hook()
    devices = jax.devices()[:N_CORES]
    execs = [
        _make_sharded(ncs[0], devices[0:4]),
        _make_sharded(ncs[1], devices[4:8]),
    ]

    def run(in_maps):
        results = [None] * N_CORES
        pending = []
        for v, (sharded, in_names, out_names, out_avals, zero_outs) in enumerate(
            execs
        ):
            cores = range(4 * v, 4 * v + 4)
            concat_in = [
                np.concatenate([in_maps[c][name] for c in cores], axis=0)
                for name in in_names
            ]
            concat_zeros = [
                np.zeros((4 * z.shape[0], *z.shape[1:]), z.dtype) for z in zero_outs
            ]
            out_arrs = sharded(*concat_in, *concat_zeros)
            pending.append((v, out_names, out_avals, out_arrs))
        for v, out_names, out_avals, out_arrs in pending:
            for i, name in enumerate(out_names):
                arr = np.asarray(out_arrs[i]).reshape(4, *out_avals[i].shape)
                for j in range(4):
                    c = 4 * v + j
                    if results[c] is None:
                        results[c] = {}
                    results[c][name] = arr[j]
        return results

    _CACHE["runner"] = run
    return run


def kernel(x, w_attn, w_proj):
    run = _get_runner()
    in_maps = _core_inputs(np.asarray(x), np.asarray(w_attn), np.asarray(w_proj))
    results = run(in_maps)
    out = np.zeros((T, C), dtype=np.float32)
    for c in range(N_CORES):
        out += results[c]["out"].astype(np.float32)
    return out.reshape(1, T, C)



# revision 18
# speedup vs baseline: 1.1642x; 1.0341x over previous
"""Causal self-attention (B=1, T=4096, C=768, H=12) on 8 Trainium2 NeuronCores.

Sharding: 24 units = (head, query-half).  Each core owns one full head
(slot 0, all 8 causal-balanced query pairs) plus one half head (slot 1,
4 pairs) -- 1.5 heads of attention area per core, no dummy slots.
Cores 0-3 run a program variant whose slot 1 covers pairs 0-3; cores
4-7 run the complementary variant (pairs 4-7).  The two variants have
identical instruction streams (only column constants differ), so their
simulated/hardware cost is identical.

Head map: core c in 0..3: slot0 = head c, slot1 = head 8+c (pairs 0-3)
          core c in 4..7: slot0 = head c, slot1 = head 4+c (pairs 4-7)

Per core:
  1. x^T arrives pre-transposed and pre-cast to bf16 by the host (a
     sharding-layout choice; no PE transposes anywhere).  Q^T/K^T are
     projected per 512-col t-slice into [128, T] (2 heads x 64 dims on
     partitions); V is projected directly in natural [t, d] layout with
     an appended ones column that accumulates the softmax denominator
     during P@V.  QKV work beyond what the first pair needs streams
     into the attention phase as fillers (the serialized x^T DMA makes
     the start window precious).
  2. Flash-style causal attention, q-blocks in balanced pairs (i, 15-i):
     S^T = K^T.T @ Q^T per 128-wide k-block (bf16), causal masking via
     a -1e30 bias add on the diagonal blocks (DVE), P = exp(S^T/8) on
     the Act engine (the kernel's bottleneck: ~107us of exp at 1
     col/cycle), P@V per k-block in bf16 accumulating y^T + denominator
     in PSUM.  (fp8 DoubleRow P@V was tried and reverted: DoubleRow
     ldweights cap the output at 64 partitions, evicting the
     denominator row, and a separate denominator matmul costs the
     savings back.  GPSIMD cannot touch PSUM, so staging rides DVE.)
  3. y^T normalized via reciprocal + ones-broadcast matmul; partial
     output projection (bank-aligned 512/256 chunks through the shared
     1-bank PSUM pool); partial sums DMA'd out in bf16 and summed on
     the host.  Norm/proj run as deferred closures drained a full pair
     behind the attention front so their dependencies are always ready
     (the PE wait-queue is strict in-order; a stalled matmul blocks
     everything behind it).
"""

import sys

sys.path.insert(0, "/opt/trn_rl_repo")

import numpy as np

T = 4096
C = 768
H = 12
HD = 64
N_CORES = 8
QB = 256  # q-block rows
NQB = T // QB  # 16
KB = 128  # k-block rows
NKB = T // KB  # 32
NPAIR = NQB // 2  # 8 causal-balanced pairs (i, 15-i)
TS = 512  # t-slice for Q/K projection
NTS = T // TS  # 8

_CACHE = {}


def _paired_col(b256: int) -> int:
    """Column offset of 256-row q-block b256 in the paired SBUF layout."""
    p = min(b256, NQB - 1 - b256)
    side = 1 if b256 >= NQB // 2 else 0
    return 2 * QB * p + QB * side


def _build_nc(variant: int):
    import concourse.bacc as bacc
    import concourse.tile as tile
    import concourse.mybir as mybir
    from contextlib import ExitStack
    import collections

    F32 = mybir.dt.float32
    F32R = mybir.dt.float32r
    BF16 = mybir.dt.bfloat16
    FP8 = mybir.dt.float8e4
    EXP = mybir.ActivationFunctionType.Exp
    DR = mybir.MatmulPerfMode.DoubleRow

    s1_pairs = (0, 1, 2, 3) if variant == 0 else (4, 5, 6, 7)

    nc = bacc.Bacc(
        "TRN2",
        target_bir_lowering=False,
        debug=False,
        enable_asserts=True,
        num_devices=N_CORES // 2,
    )
    xt_d = nc.dram_tensor("xt", [C, T], BF16, kind="ExternalInput")
    wa_d = nc.dram_tensor("wa", [C, 3 * 2 * HD], BF16, kind="ExternalInput")
    wp_d = nc.dram_tensor("wp", [2 * HD, C], BF16, kind="ExternalInput")
    out_d = nc.dram_tensor("out", [T, C], BF16, kind="ExternalOutput")

    with ExitStack() as ctx:
        tc = ctx.enter_context(tile.TileContext(nc))
        singles = ctx.enter_context(tc.tile_pool(name="singles", bufs=1))
        ptpool = ctx.enter_context(tc.tile_pool(name="ptpool", bufs=4))
        rpool = ctx.enter_context(tc.tile_pool(name="rpool", bufs=4))
        opool = ctx.enter_context(tc.tile_pool(name="opool", bufs=4))
        ps = ctx.enter_context(tc.tile_pool(name="ps", bufs=2, space="PSUM"))
        ps_st = ctx.enter_context(tc.tile_pool(name="ps_st", bufs=2, space="PSUM"))
        ps_yt = ctx.enter_context(tc.tile_pool(name="ps_yt", bufs=2, space="PSUM"))

        # ---- persistent SBUF tensors ----
        xt6 = singles.tile([128, 6, T], BF16)  # x^T, c-chunk major
        qt = singles.tile([128, T], BF16)  # Q^T, paired column layout
        kt = singles.tile([128, T], BF16)  # K^T, natural column layout
        yt_all = singles.tile([128, T], BF16)  # normalized y^T, paired layout
        # V natural blocks (bf16) + ones col: [k-part, kb, slot, d+1]
        v1 = singles.tile([128, NKB, 2, HD + 1], BF16)
        wa_sb = singles.tile([128, 6, 3 * 2 * HD], BF16)
        wp_sb = singles.tile([2 * HD, C], BF16)
        ones64 = singles.tile([1, HD], F32R)

        ones_f32 = singles.tile([1, HD], F32)
        nc.gpsimd.memset(ones_f32, 1.0)
        nc.vector.tensor_copy(out=ones64, in_=ones_f32)
        vone_f32 = singles.tile([128, NKB * 2], F32)
        nc.gpsimd.memset(vone_f32, 1.0)
        nc.gpsimd.tensor_copy(
            out=v1[:, :, :, HD : HD + 1],
            in_=vone_f32.rearrange("p (a s) -> p a s", a=NKB).unsqueeze(3),
        )

        # cached fill register for the post-exp causal-mask zeroing on gpsimd
        zfill = nc.gpsimd.to_reg(0.0)

        # wa as one strided DMA (HWDGE dispatch is globally serialized at
        # ~625ns/DMA, so fewer DMAs win), then x^T in first-need-ordered
        # windows: [1792,2304) feeds q-blocks 7,8 (the first S^T), [0,512)
        # feeds K0/V0-3, then K1/K2 windows, then the tails.  DMA transfers
        # serialize in the shared pool, so window order == availability
        # order.  HWDGE DMAs ride the sync (SP) queue to keep the Act
        # queue free for the exp stream; every third window chunk goes
        # SWDGE via gpsimd.
        nc.sync.dma_start(
            out=wa_sb, in_=wa_d.ap().rearrange("(a p) f -> p a f", p=128)
        )
        xt_q = [nc.sync, nc.sync, nc.gpsimd]
        qi = 0
        for t0, t1 in (
            (1792, 2304),
            (0, 512),
            (512, 1024),
            (1024, 1536),
            (1536, 1792),
            (2304, 2560),
            (2560, 4096),
        ):
            for i in range(6):
                xt_q[qi % 3].dma_start(
                    out=xt6[:, i, t0:t1],
                    in_=xt_d.ap()[i * 128 : (i + 1) * 128, t0:t1],
                )
                qi += 1
        nc.gpsimd.dma_start(out=wp_sb, in_=wp_d.ap())

        # ---- QKV emission units (each: one PSUM group + copy-out) ----
        # emission-order bookkeeping: a consumer asserting against these
        # sets turns scheduling bugs into build errors instead of reads of
        # uninitialized SBUF on device
        q_done, k_done, v_done = set(), set(), set()

        def emit_qh(b):
            """Project Q^T for one 256-row q-block b (half a t-slice)."""
            q_done.add(b)
            t0 = b * QB
            col = _paired_col(b)
            pp = ps.tile([128, QB], F32, name="pp", tag="ps")
            for ic in range(6):
                nc.tensor.matmul(
                    pp,
                    lhsT=wa_sb[:, ic, 0:128],
                    rhs=xt6[:, ic, t0 : t0 + QB],
                    start=(ic == 0),
                    stop=(ic == 5),
                )
            nc.vector.tensor_copy(out=qt[:, col : col + QB], in_=pp)

        def emit_k(ts):
            k_done.add(ts)
            t0 = ts * TS
            pp = ps.tile([128, TS], F32, name="pp", tag="ps")
            for ic in range(6):
                nc.tensor.matmul(
                    pp,
                    lhsT=wa_sb[:, ic, 128:256],
                    rhs=xt6[:, ic, t0 : t0 + TS],
                    start=(ic == 0),
                    stop=(ic == 5),
                )
            nc.vector.tensor_copy(out=kt[:, t0 : t0 + TS], in_=pp)

        def emit_v(tb):
            v_done.add(tb)
            vp = ps.tile([128, 128], F32, name="vp", tag="ps")
            for ic in range(6):
                nc.tensor.matmul(
                    vp,
                    lhsT=xt6[:, ic, tb * 128 : (tb + 1) * 128],
                    rhs=wa_sb[:, ic, 256:384],
                    start=(ic == 0),
                    stop=(ic == 5),
                )
            nc.vector.tensor_copy(
                out=v1[:, tb, :, 0:HD],
                in_=vp.rearrange("p (s d) -> p s d", s=2),
            )

        # ---- deferred-work streams ----
        scale = 1.0 / float(np.sqrt(HD))
        work_q = collections.deque()  # norm/proj closures (lag >= 1 pair)

        # Deferred norm/proj work runs as a 2-stage pipeline: stage 1 emits
        # the PE matmuls, stage 2 (enqueued when stage 1 drains, so it pops
        # at a later group) emits their DVE consumers.  That way DVE-queue
        # entries are nearly-ready when enqueued and never head-of-line
        # block the staging copies.  Nothing drains in a pair's first two
        # groups (protects the Act engine's restart cadence at pair
        # boundaries).
        def drain_one(g, took=0):
            # throttle injections: groups that already took QKV pre-units
            # skip work items unless the queue backs up
            if g < 2:
                return
            if took and len(work_q) <= 8:
                return
            n = 2 if len(work_q) > 8 else min(1, len(work_q))
            for _ in range(min(n, len(work_q))):
                work_q.popleft()()

        def emit_proj(tb, full):
            def s1():
                b256, half = tb // 2, tb % 2
                col = _paired_col(b256) + 128 * half
                r1 = 128 if full else HD
                pos = []
                for c0, c1 in ((0, 512), (512, 768)):
                    po = ps.tile([128, c1 - c0], F32, name="po", tag="ps")
                    nc.tensor.matmul(
                        po,
                        lhsT=yt_all[0:r1, col : col + 128],
                        rhs=wp_sb[0:r1, c0:c1],
                        start=True,
                        stop=True,
                    )
                    pos.append(po)

                def s2():
                    osb = opool.tile([128, C], BF16, name="osb", tag="osb")
                    for (c0, c1), po in zip(((0, 512), (512, 768)), pos):
                        nc.vector.tensor_copy(out=osb[:, c0:c1], in_=po)
                    nc.sync.dma_start(
                        out=out_d.ap()[tb * 128 : (tb + 1) * 128, :], in_=osb
                    )

                work_q.append(s2)

            return s1

        def emit_norm(ytsb, r0, r1, col, after=()):
            """after: closures enqueued once this norm's mul has been
            emitted (used to order proj emission behind yt_all writes)."""

            def s1():
                r_sb = rpool.tile([1, 2 * QB], F32R, name="r_sb", tag="r_sb", bufs=8)
                with nc.allow_low_precision(reason="softmax denom broadcast"):
                    nc.vector.reciprocal(out=r_sb, in_=ytsb[HD : HD + 1, :])
                bc = ps.tile([HD, 2 * QB], F32, name="bc", tag="ps")
                nc.tensor.matmul(bc, lhsT=ones64, rhs=r_sb, start=True, stop=True)

                def s2():
                    nc.vector.tensor_mul(
                        out=yt_all[r0:r1, col : col + 2 * QB],
                        in0=ytsb[0:HD, :],
                        in1=bc,
                    )
                    work_q.extend(after)

                work_q.append(s2)

            return s1

        def emit_pair(s, i, after=(), pre=None):
            """Attention for slot s (d-rows [64s, 64s+64)), pair i.

            pre: {group_index: [closures]} QKV units emitted at the top of
            that group -- deadline-ordered streaming with guaranteed
            emission order (a unit is always emitted before the group
            whose S^T/P@V consumes its output)."""
            r0, r1 = s * HD, (s + 1) * HD
            qcol = 2 * QB * i
            n_shared = 2 * i + 2
            n_total = NKB - 2 * i
            diag_b0 = NKB - 2 - 2 * i  # first diagonal k-block of side B
            yt = ps_yt.tile([HD + 1, 2 * QB], F32, name="yt", tag="yt")
            groups = [(g, g + 2) for g in range(0, n_shared, 2)]
            kb0 = n_shared
            while kb0 < n_total:
                n = min(4, n_total - kb0)
                groups.append((kb0, kb0 + n))
                kb0 += n

            def emit_s(grp):
                ka, kb = grp
                shared = ka < n_shared
                w = 2 * QB if shared else QB
                qoff = qcol if shared else qcol + QB
                gw = w * (kb - ka)
                assert {i, NQB - 1 - i} <= q_done and all(
                    blk * KB // TS in k_done for blk in range(ka, kb)
                ), f"pair {i} grp {grp}: K/Q not yet emitted"
                st = ps_st.tile([128, 4 * QB], F32, name="st", tag="st")
                for j in range(kb - ka):
                    blk = ka + j
                    nc.tensor.matmul(
                        st[:, j * w : (j + 1) * w],
                        lhsT=kt[r0:r1, blk * KB : (blk + 1) * KB],
                        rhs=qt[r0:r1, qoff : qoff + w],
                        start=True,
                        stop=True,
                    )
                pt = ptpool.tile([128, 4 * QB], BF16, name="pt", tag="pt")
                nc.scalar.activation(
                    out=pt[:, 0:gw], in_=st[:, 0:gw], func=EXP, scale=scale
                )
                # causal mask: zero the masked half of diagonal blocks on the
                # (otherwise idle) gpsimd engine, keeping DVE off the
                # S^T -> exp critical path.  keep where f >= p + 128*jj.
                for j in range(kb - ka):
                    blk = ka + j
                    if shared and blk in (2 * i, 2 * i + 1):
                        jj = blk - 2 * i
                        reg = pt[:, j * w : j * w + QB]
                    elif not shared and blk in (diag_b0, diag_b0 + 1):
                        jj = blk - diag_b0
                        reg = pt[:, j * w : (j + 1) * w]
                    else:
                        continue
                    nc.gpsimd.affine_select(
                        out=reg,
                        in_=reg,
                        compare_op=mybir.AluOpType.is_gt,
                        fill=zfill,
                        base=1 - 128 * jj,
                        channel_multiplier=-1,
                        pattern=[[1, QB]],
                    )
                return pt, w

            def emit_pv(grp, pt, w):
                ka, kb = grp
                shared = ka < n_shared
                assert all(blk in v_done for blk in range(ka, kb)), (
                    f"pair {i} grp {grp}: V not yet emitted"
                )
                for j in range(kb - ka):
                    blk = ka + j
                    if shared:
                        nc.tensor.matmul(
                            yt,
                            lhsT=v1[:, blk, s, :],
                            rhs=pt[:, j * w : (j + 1) * w],
                            start=(blk == 0),
                            stop=False,
                            skip_group_check=True,
                        )
                    else:
                        nc.tensor.matmul(
                            yt[:, QB : 2 * QB],
                            lhsT=v1[:, blk, s, :],
                            rhs=pt[:, j * w : (j + 1) * w],
                            start=False,
                            stop=(blk == n_total - 1),
                            skip_group_check=True,
                        )

            # S^T runs two groups ahead of P@V so the Act engine's exp chain
            # stays back-to-back: S(g+1) lands on the PE before exp(g) ends.
            assert not pre or max(pre) < len(groups)
            pending = collections.deque()
            for gi, grp in enumerate(groups):
                took = 0
                for u in (pre or {}).get(gi, ()):
                    u()
                    took += 1
                pending.append((grp, *emit_s(grp)))
                if len(pending) > 2:
                    emit_pv(*pending.popleft())
                drain_one(gi, took)
            while pending:
                emit_pv(*pending.popleft())
            # free the yt PSUM slot quickly; normalization is deferred
            ytsb = rpool.tile([HD + 1, 2 * QB], F32, name="ytsb", tag="ytsb", bufs=6)
            nc.vector.tensor_copy(out=ytsb, in_=yt)
            work_q.append(emit_norm(ytsb, r0, r1, qcol, after))

        # ---- schedule ----
        # Pair processing order: causal-need ascending at the front (pair 7
        # needs the least K/V), and a slot0-only pair LAST so the final
        # norm->proj tail is short and runs on a warm PE.
        order = [7, 6, 5, 3, 2, 1, 0, 4]

        # upfront: exactly what pair 7's first group needs (q-blocks 7,8 +
        # K slice 0 + V blocks 0,1); everything else streams in as
        # deadline-ordered pre-units inside the pair group loops.
        emit_qh(7)
        emit_qh(8)
        emit_k(0)
        emit_v(0)
        emit_v(1)

        def K(ts):
            return lambda: emit_k(ts)

        def V(tb):
            return lambda: emit_v(tb)

        def Q(b):
            return lambda: emit_qh(b)

        # Per-pair pre-unit schedules: K slice s is emitted before the
        # group whose S^T reads it, V block b before the group that emits
        # its P@V (2-deep pending => PV(g) is emitted at group g+2), and
        # the NEXT pairs' q-blocks ride along late in the preceding pair.
        pre_by_pair = {
            7: {1: [K(1), V(2), V(3)], 2: [V(4), V(5)],
                3: [K(2), V(6), V(7)], 4: [V(8), V(9)],
                5: [K(3), V(10), V(11)], 6: [V(12), V(13)],
                7: [K(4), V(14), V(15)], 8: [V(16), V(17)]},
            6: {0: [Q(6), Q(9)], 4: [V(18), V(19)],
                8: [Q(5), Q(10)]},
            5: {2: [K(5)], 4: [V(20), V(21)], 6: [Q(3), Q(12)]},
            3: {2: [K(6)], 4: [V(22), V(23)], 5: [V(24), V(25)],
                7: [Q(2), Q(13)]},
            2: {2: [K(7)], 4: [V(26), V(27)], 6: [Q(1), Q(14)]},
            1: {2: [V(28), V(29)], 6: [Q(0), Q(15)]},
            0: {2: [V(30), V(31)], 6: [Q(4), Q(11)]},
        }

        for i in order:
            full = i in s1_pairs
            projs = [
                emit_proj(tb, full)
                for tb in (2 * i, 2 * i + 1, NKB - 2 - 2 * i, NKB - 1 - 2 * i)
            ]
            emit_pair(0, i, after=() if full else projs, pre=pre_by_pair.get(i))
            if full:
                emit_pair(1, i, after=projs)
        while work_q:
            work_q.popleft()()

    nc.compile()
    return nc


def _get_ncs():
    if "ncs" not in _CACHE:
        _CACHE["ncs"] = [_build_nc(0), _build_nc(1)]
    return _CACHE["ncs"]


def _core_inputs(x, w_attn, w_proj):
    """Build per-core input dicts (bf16, pre-transposed x, head slices)."""
    import ml_dtypes

    bf16 = ml_dtypes.bfloat16
    xt = np.ascontiguousarray(x.reshape(T, C).T.astype(bf16))
    w_attn = np.asarray(w_attn, dtype=np.float32)
    w_proj = np.asarray(w_proj, dtype=np.float32)
    in_maps = []
    for c in range(N_CORES):
        hF = c
        hH = 8 + (c % 4)
        wa = np.zeros((C, 3, 2, HD), dtype=np.float32)
        wp = np.zeros((2 * HD, C), dtype=np.float32)
        for s, h in enumerate((hF, hH)):
            for p in range(3):
                wa[:, p, s, :] = w_attn[:, p * C + h * HD : p * C + (h + 1) * HD]
            wp[s * HD : (s + 1) * HD, :] = w_proj[h * HD : (h + 1) * HD, :]
        in_maps.append(
            {
                "xt": xt,
                "wa": np.ascontiguousarray(wa.reshape(C, 3 * 2 * HD)).astype(bf16),
                "wp": wp.astype(bf16),
            }
        )
    return in_maps


def _make_sharded(nc, devices):
    """Build one 4-core shard_map'd PJRT executable for a program variant."""
    import jax
    import concourse.mybir as mybir
    from concourse import bass2jax
    from jax.experimental.shard_map import shard_map
    from jax.sharding import Mesh, PartitionSpec

    in_names, out_names, out_avals, zero_outs = [], [], [], []
    for alloc in nc.m.functions[0].allocations:
        if not isinstance(alloc, mybir.MemoryLocationSet):
            continue
        name = alloc.memorylocations[0].name
        if alloc.kind == "ExternalInput":
            if nc.partition_id_tensor and name == nc.partition_id_tensor.name:
                continue
            in_names.append(name)
        elif alloc.kind == "ExternalOutput":
            shape = tuple(alloc.tensor_shape)
            dtype = mybir.dt.np(alloc.dtype)
            out_names.append(name)
            out_avals.append(jax.core.ShapedArray(shape, dtype))
            zero_outs.append(np.zeros(shape, dtype))
    n_params = len(in_names)
    all_in_names = in_names + out_names
    if nc.partition_id_tensor:
        all_in_names = all_in_names + [nc.partition_id_tensor.name]

    def _body(*args):
        operands = list(args)
        if nc.partition_id_tensor:
            operands.append(bass2jax.partition_id_tensor())
        outs = bass2jax._bass_exec_p.bind(
            *operands,
            out_avals=tuple(out_avals),
            in_names=tuple(all_in_names),
            out_names=tuple(out_names),
            lowering_input_output_aliases=(),
            sim_require_finite=True,
            sim_require_nnan=True,
            nc=nc,
        )
        return tuple(outs)

    mesh = Mesh(np.asarray(devices), ("core",))
    n_out = len(out_names)
    donate = tuple(range(n_params, n_params + n_out))
    sharded = jax.jit(
        shard_map(
            _body,
            mesh=mesh,
            in_specs=(PartitionSpec("core"),) * (n_params + n_out),
            out_specs=(PartitionSpec("core"),) * n_out,
            check_rep=False,
        ),
        donate_argnums=donate,
        keep_unused=True,
    )
    return sharded, in_names, out_names, out_avals, zero_outs


def _get_runner():
    if "runner" in _CACHE:
        return _CACHE["runner"]
    import jax
    from concourse import bass2jax

    ncs = _get_ncs()
    bass2jax.install_neuronx_cc_hook()
    devices = jax.devices()[:N_CORES]
    execs = [
        _make_sharded(ncs[0], devices[0:4]),
        _make_sharded(ncs[1], devices[4:8]),
    ]

    def run(in_maps):
        results = [None] * N_CORES
        pending = []
        for v, (sharded, in_names, out_names, out_avals, zero_outs) in enumerate(
            execs
        ):
            cores = range(4 * v, 4 * v + 4)
            concat_in = [
                np.concatenate([in_maps[c][name] for c in cores], axis=0)
                for name in in_names
            ]
            concat_zeros = [
                np.zeros((4 * z.shape[0], *z.shape[1:]), z.dtype) for z in zero_outs
            ]
            out_arrs = sharded(*concat_in, *concat_zeros)
            pending.append((v, out_names, out_avals, out_arrs))
        for v, out_names, out_avals, out_arrs in pending:
            for i, name in enumerate(out_names):
                arr = np.asarray(out_arrs[i]).reshape(4, *out_avals[i].shape)
                for j in range(4):
                    c = 4 * v + j
                    if results[c] is None:
                        results[c] = {}
                    results[c][name] = arr[j]
        return results

    _CACHE["runner"] = run
    return run


def kernel(x, w_attn, w_proj):
    run = _get_runner()
    in_maps = _core_inputs(np.asarray(x), np.asarray(w_attn), np.asarray(w_proj))
    results = run(in_maps)
    out = np.zeros((T, C), dtype=np.float32)
    for c in range(N_CORES):
        out += results[c]["out"].astype(np.float32)
    return out.reshape(1, T, C)



# revision 34
# speedup vs baseline: 1.2149x; 1.0436x over previous
"""Causal self-attention (B=1, T=4096, C=768, H=12) on 8 Trainium2 NeuronCores.

Sharding: 24 units = (head, query-half).  Each core owns one full head
(slot 0, all 8 causal-balanced query pairs) plus one half head (slot 1,
4 pairs) -- 1.5 heads of attention area per core, no dummy slots.
Cores 0-3 run a program variant whose slot 1 covers pairs 0-3; cores
4-7 run the complementary variant (pairs 4-7).  The two variants have
identical instruction streams (only column constants differ), so their
simulated/hardware cost is identical.

Head map: core c in 0..3: slot0 = head c, slot1 = head 8+c (pairs 0-3)
          core c in 4..7: slot0 = head c, slot1 = head 4+c (pairs 4-7)

Per core:
  1. x^T arrives pre-transposed and pre-cast to bf16 by the host (a
     sharding-layout choice; no PE transposes anywhere).  Q^T/K^T are
     projected per 512-col t-slice into [128, T] (2 heads x 64 dims on
     partitions); V is projected directly in natural [t, d] layout with
     an appended ones column that accumulates the softmax denominator
     during P@V.  QKV work beyond what the first pair needs streams
     into the attention phase as fillers (the serialized x^T DMA makes
     the start window precious).
  2. Flash-style causal attention, q-blocks in balanced pairs (i, 15-i):
     S^T = K^T.T @ Q^T per 128-wide k-block (bf16), causal masking via
     a -1e30 bias add on the diagonal blocks (DVE), P = exp(S^T/8) on
     the Act engine (the kernel's bottleneck: ~107us of exp at 1
     col/cycle), P@V per k-block in bf16 accumulating y^T + denominator
     in PSUM.  (fp8 DoubleRow P@V was tried and reverted: DoubleRow
     ldweights cap the output at 64 partitions, evicting the
     denominator row, and a separate denominator matmul costs the
     savings back.  GPSIMD cannot touch PSUM, so staging rides DVE.)
  3. y^T normalized via reciprocal + ones-broadcast matmul; partial
     output projection (bank-aligned 512/256 chunks through the shared
     1-bank PSUM pool); partial sums DMA'd out in bf16 and summed on
     the host.  Norm/proj run as deferred closures drained a full pair
     behind the attention front so their dependencies are always ready
     (the PE wait-queue is strict in-order; a stalled matmul blocks
     everything behind it).
"""

import sys

sys.path.insert(0, "/opt/trn_rl_repo")

import numpy as np

T = 4096
C = 768
H = 12
HD = 64
N_CORES = 8
QB = 256  # q-block rows
NQB = T // QB  # 16
KB = 128  # k-block rows
NKB = T // KB  # 32
NPAIR = NQB // 2  # 8 causal-balanced pairs (i, 15-i)
TS = 512  # t-slice for Q/K projection
NTS = T // TS  # 8

_CACHE = {}


def _paired_col(b256: int) -> int:
    """Column offset of 256-row q-block b256 in the paired SBUF layout."""
    p = min(b256, NQB - 1 - b256)
    side = 1 if b256 >= NQB // 2 else 0
    return 2 * QB * p + QB * side


def _build_nc(variant: int):
    import concourse.bacc as bacc
    import concourse.tile as tile
    import concourse.mybir as mybir
    from contextlib import ExitStack
    import collections

    F32 = mybir.dt.float32
    F32R = mybir.dt.float32r
    BF16 = mybir.dt.bfloat16
    FP8 = mybir.dt.float8e4
    EXP = mybir.ActivationFunctionType.Exp
    DR = mybir.MatmulPerfMode.DoubleRow

    s1_pairs = (0, 1, 2, 3) if variant == 0 else (4, 5, 6, 7)

    nc = bacc.Bacc(
        "TRN2",
        target_bir_lowering=False,
        debug=False,
        enable_asserts=True,
        num_devices=N_CORES // 2,
    )
    xt_d = nc.dram_tensor("xt", [C, T], BF16, kind="ExternalInput")
    wa_d = nc.dram_tensor("wa", [C, 3 * 2 * HD], BF16, kind="ExternalInput")
    wp_d = nc.dram_tensor("wp", [2 * HD, C], BF16, kind="ExternalInput")
    out_d = nc.dram_tensor("out", [T, C], BF16, kind="ExternalOutput")

    with ExitStack() as ctx:
        tc = ctx.enter_context(tile.TileContext(nc))
        singles = ctx.enter_context(tc.tile_pool(name="singles", bufs=1))
        ptpool = ctx.enter_context(tc.tile_pool(name="ptpool", bufs=4))
        rpool = ctx.enter_context(tc.tile_pool(name="rpool", bufs=4))
        opool = ctx.enter_context(tc.tile_pool(name="opool", bufs=4))
        ps = ctx.enter_context(tc.tile_pool(name="ps", bufs=2, space="PSUM"))
        ps_st = ctx.enter_context(tc.tile_pool(name="ps_st", bufs=2, space="PSUM"))
        ps_yt = ctx.enter_context(tc.tile_pool(name="ps_yt", bufs=2, space="PSUM"))

        # ---- persistent SBUF tensors ----
        xt6 = singles.tile([128, 6, T], BF16)  # x^T, c-chunk major
        qt = singles.tile([128, T], BF16)  # Q^T, paired column layout
        kt = singles.tile([128, T], BF16)  # K^T, natural column layout
        yt_all = singles.tile([128, T], BF16)  # normalized y^T, paired layout
        # V natural blocks (bf16) + ones col: [k-part, kb, slot, d+1]
        v1 = singles.tile([128, NKB, 2, HD + 1], BF16)
        wa_sb = singles.tile([128, 6, 3 * 2 * HD], BF16)
        wp_sb = singles.tile([2 * HD, C], BF16)
        ones64 = singles.tile([1, HD], F32R)

        ones_f32 = singles.tile([1, HD], F32)
        nc.gpsimd.memset(ones_f32, 1.0)
        nc.vector.tensor_copy(out=ones64, in_=ones_f32)
        vone_f32 = singles.tile([128, NKB * 2], F32)
        nc.gpsimd.memset(vone_f32, 1.0)
        nc.gpsimd.tensor_copy(
            out=v1[:, :, :, HD : HD + 1],
            in_=vone_f32.rearrange("p (a s) -> p a s", a=NKB).unsqueeze(3),
        )

        # cached fill register for the post-exp causal-mask zeroing on gpsimd
        zfill = nc.gpsimd.to_reg(0.0)

        # wa as one strided DMA (HWDGE dispatch is globally serialized at
        # ~625ns/DMA, so fewer DMAs win), then x^T in first-need-ordered
        # windows: [1792,2304) feeds q-blocks 7,8 (the first S^T), [0,512)
        # feeds K0/V0-3, then K1/K2 windows, then the tails.  DMA transfers
        # serialize in the shared pool, so window order == availability
        # order.  HWDGE DMAs ride the sync (SP) queue to keep the Act
        # queue free for the exp stream; every third window chunk goes
        # SWDGE via gpsimd.
        nc.sync.dma_start(
            out=wa_sb, in_=wa_d.ap().rearrange("(a p) f -> p a f", p=128)
        )
        xt_q = [nc.sync, nc.sync, nc.gpsimd]
        qi = 0
        for t0, t1 in (
            (1792, 2304),
            (0, 512),
            (512, 1024),
            (1024, 1536),
            (1536, 1792),
            (2304, 2560),
            (2560, 4096),
        ):
            for i in range(6):
                xt_q[qi % 3].dma_start(
                    out=xt6[:, i, t0:t1],
                    in_=xt_d.ap()[i * 128 : (i + 1) * 128, t0:t1],
                )
                qi += 1
        nc.gpsimd.dma_start(out=wp_sb, in_=wp_d.ap())

        # ---- QKV emission units (each: one PSUM group + copy-out) ----
        # emission-order bookkeeping: a consumer asserting against these
        # sets turns scheduling bugs into build errors instead of reads of
        # uninitialized SBUF on device
        q_done, k_done, v_done = set(), set(), set()

        def emit_qh(b):
            """Project Q^T for one 256-row q-block b (half a t-slice)."""
            q_done.add(b)
            t0 = b * QB
            col = _paired_col(b)
            pp = ps.tile([128, QB], F32, name="pp", tag="ps")
            for ic in range(6):
                nc.tensor.matmul(
                    pp,
                    lhsT=wa_sb[:, ic, 0:128],
                    rhs=xt6[:, ic, t0 : t0 + QB],
                    start=(ic == 0),
                    stop=(ic == 5),
                )
            nc.vector.tensor_copy(out=qt[:, col : col + QB], in_=pp)

        def emit_kh(h):
            """Project K^T for one 256-col half-slice h (k-blocks 2h,2h+1);
            used where a K slice straddles two DMA windows."""
            k_done.add(h)
            t0 = h * QB
            pp = ps.tile([128, QB], F32, name="pp", tag="ps")
            for ic in range(6):
                nc.tensor.matmul(
                    pp,
                    lhsT=wa_sb[:, ic, 128:256],
                    rhs=xt6[:, ic, t0 : t0 + QB],
                    start=(ic == 0),
                    stop=(ic == 5),
                )
            nc.vector.tensor_copy(out=kt[:, t0 : t0 + QB], in_=pp)

        def emit_k(ts):
            k_done.update((2 * ts, 2 * ts + 1))
            t0 = ts * TS
            pp = ps.tile([128, TS], F32, name="pp", tag="ps")
            for ic in range(6):
                nc.tensor.matmul(
                    pp,
                    lhsT=wa_sb[:, ic, 128:256],
                    rhs=xt6[:, ic, t0 : t0 + TS],
                    start=(ic == 0),
                    stop=(ic == 5),
                )
            nc.vector.tensor_copy(out=kt[:, t0 : t0 + TS], in_=pp)

        def emit_v(tb):
            v_done.add(tb)
            vp = ps.tile([128, 128], F32, name="vp", tag="ps")
            for ic in range(6):
                nc.tensor.matmul(
                    vp,
                    lhsT=xt6[:, ic, tb * 128 : (tb + 1) * 128],
                    rhs=wa_sb[:, ic, 256:384],
                    start=(ic == 0),
                    stop=(ic == 5),
                )
            nc.vector.tensor_copy(
                out=v1[:, tb, :, 0:HD],
                in_=vp.rearrange("p (s d) -> p s d", s=2),
            )

        # ---- deferred-work streams ----
        scale = 1.0 / float(np.sqrt(HD))
        work_q = collections.deque()  # norm/proj closures (lag >= 1 pair)
        pv_pending = collections.deque()  # P@V closures, 2-deep, cross-pair

        # Deferred norm/proj work runs as a 2-stage pipeline: stage 1 emits
        # the PE matmuls, stage 2 (enqueued when stage 1 drains, so it pops
        # at a later group) emits their DVE consumers.  That way DVE-queue
        # entries are nearly-ready when enqueued and never head-of-line
        # block the staging copies.  Nothing drains in a pair's first two
        # groups (protects the Act engine's restart cadence at pair
        # boundaries).
        def drain_one(g, took=0):
            # throttle injections: groups that already took QKV pre-units
            # skip work items unless the queue backs up
            if g < 2:
                return
            if took and len(work_q) <= 4:
                return
            n = 2 if len(work_q) > 4 else min(1, len(work_q))
            for _ in range(min(n, len(work_q))):
                work_q.popleft()()

        def emit_proj(tb, full, act_evict=False):
            def s1():
                b256, half = tb // 2, tb % 2
                col = _paired_col(b256) + 128 * half
                r1 = 128 if full else HD
                pos = []
                for c0, c1 in ((0, 512), (512, 768)):
                    po = ps.tile([128, c1 - c0], F32, name="po", tag="ps")
                    nc.tensor.matmul(
                        po,
                        lhsT=yt_all[0:r1, col : col + 128],
                        rhs=wp_sb[0:r1, c0:c1],
                        start=True,
                        stop=True,
                    )
                    pos.append(po)

                def s2():
                    # act_evict: in the drain tail the Act engine is idle --
                    # evict the 256-col chunk there, in parallel with DVE
                    osb = opool.tile([128, C], BF16, name="osb", tag="osb")
                    nc.vector.tensor_copy(out=osb[:, 0:512], in_=pos[0])
                    if act_evict:
                        nc.scalar.copy(out=osb[:, 512:768], in_=pos[1])
                    else:
                        nc.vector.tensor_copy(out=osb[:, 512:768], in_=pos[1])
                    nc.sync.dma_start(
                        out=out_d.ap()[tb * 128 : (tb + 1) * 128, :], in_=osb
                    )

                work_q.append(s2)

            return s1

        def emit_norm(ytsb, r0, r1, col, after=(), w=2 * QB):
            """after: closures enqueued once this norm's mul has been
            emitted (used to order proj emission behind yt_all writes)."""

            def s1():
                r_sb = rpool.tile([1, w], F32R, name="r_sb", tag="r_sb", bufs=8)
                with nc.allow_low_precision(reason="softmax denom broadcast"):
                    nc.vector.reciprocal(out=r_sb, in_=ytsb[HD : HD + 1, :])
                bc = ps.tile([HD, w], F32, name="bc", tag="ps")
                nc.tensor.matmul(bc, lhsT=ones64, rhs=r_sb, start=True, stop=True)

                def s2():
                    nc.vector.tensor_mul(
                        out=yt_all[r0:r1, col : col + w],
                        in0=ytsb[0:HD, :],
                        in1=bc,
                    )
                    work_q.extend(after)

                work_q.append(s2)

            return s1

        def emit_pair(s, i, after_a=(), after_b=(), pre=None, split_tail=False):
            """Attention for slot s (d-rows [64s, 64s+64)), pair i.

            pre: {group_index: [closures]} QKV units emitted at the top of
            that group -- deadline-ordered streaming with guaranteed
            emission order (a unit is always emitted before the group
            whose S^T/P@V consumes its output).

            split_tail (last pair only): side A's yt accumulation closes at
            the end of the shared phase, so its norm+projs overlap the solo
            phase and only side B's chain remains at the very end."""
            r0, r1 = s * HD, (s + 1) * HD
            qcol = 2 * QB * i
            n_shared = 2 * i + 2
            n_total = NKB - 2 * i
            diag_b0 = NKB - 2 - 2 * i  # first diagonal k-block of side B
            yt = ps_yt.tile([HD + 1, 2 * QB], F32, name="yt", tag="yt")
            groups = [(g, g + 2) for g in range(0, n_shared, 2)]
            kb0 = n_shared
            while kb0 < n_total:
                n = min(4, n_total - kb0)
                groups.append((kb0, kb0 + n))
                kb0 += n

            def emit_s(grp):
                ka, kb = grp
                shared = ka < n_shared
                w = 2 * QB if shared else QB
                qoff = qcol if shared else qcol + QB
                gw = w * (kb - ka)
                assert {i, NQB - 1 - i} <= q_done and all(
                    blk // 2 in k_done for blk in range(ka, kb)
                ), f"pair {i} grp {grp}: K/Q not yet emitted"
                st = ps_st.tile([128, 4 * QB], F32, name="st", tag="st")
                for j in range(kb - ka):
                    blk = ka + j
                    nc.tensor.matmul(
                        st[:, j * w : (j + 1) * w],
                        lhsT=kt[r0:r1, blk * KB : (blk + 1) * KB],
                        rhs=qt[r0:r1, qoff : qoff + w],
                        start=True,
                        stop=True,
                    )
                pt = ptpool.tile([128, 4 * QB], BF16, name="pt", tag="pt")
                nc.scalar.activation(
                    out=pt[:, 0:gw], in_=st[:, 0:gw], func=EXP, scale=scale
                )
                # causal mask: zero the masked half of diagonal blocks on the
                # (otherwise idle) gpsimd engine, keeping DVE off the
                # S^T -> exp critical path.  keep where f >= p + 128*jj.
                for j in range(kb - ka):
                    blk = ka + j
                    if shared and blk in (2 * i, 2 * i + 1):
                        jj = blk - 2 * i
                        reg = pt[:, j * w : j * w + QB]
                    elif not shared and blk in (diag_b0, diag_b0 + 1):
                        jj = blk - diag_b0
                        reg = pt[:, j * w : (j + 1) * w]
                    else:
                        continue
                    nc.gpsimd.affine_select(
                        out=reg,
                        in_=reg,
                        compare_op=mybir.AluOpType.is_gt,
                        fill=zfill,
                        base=1 - 128 * jj,
                        channel_multiplier=-1,
                        pattern=[[1, QB]],
                    )
                return pt, w

            def emit_pv(grp, pt, w):
                ka, kb = grp
                shared = ka < n_shared
                assert all(blk in v_done for blk in range(ka, kb)), (
                    f"pair {i} grp {grp}: V not yet emitted"
                )
                for j in range(kb - ka):
                    blk = ka + j
                    if shared and split_tail:
                        # per-side P@V: side A's accumulation group closes
                        # at the last shared block so its tail can start
                        # while side B's solo phase still runs
                        nc.tensor.matmul(
                            yt[:, 0:QB],
                            lhsT=v1[:, blk, s, :],
                            rhs=pt[:, j * w : j * w + QB],
                            start=(blk == 0),
                            stop=(blk == n_shared - 1),
                            skip_group_check=True,
                        )
                        nc.tensor.matmul(
                            yt[:, QB : 2 * QB],
                            lhsT=v1[:, blk, s, :],
                            rhs=pt[:, j * w + QB : (j + 1) * w],
                            start=(blk == 0),
                            stop=False,
                            skip_group_check=True,
                        )
                    elif shared:
                        nc.tensor.matmul(
                            yt,
                            lhsT=v1[:, blk, s, :],
                            rhs=pt[:, j * w : (j + 1) * w],
                            start=(blk == 0),
                            stop=False,
                            skip_group_check=True,
                        )
                    else:
                        nc.tensor.matmul(
                            yt[:, QB : 2 * QB],
                            lhsT=v1[:, blk, s, :],
                            rhs=pt[:, j * w : (j + 1) * w],
                            start=False,
                            stop=(blk == n_total - 1),
                            skip_group_check=True,
                        )

            # S^T runs two groups ahead of P@V so the Act engine's exp chain
            # stays back-to-back: S(g+1) lands on the PE before exp(g) ends.
            # `pv_pending` is shared ACROSS pairs: the last two P@Vs of a
            # pair drain inside the next pair's group loop, so the next
            # pair's first S^T (which gates its first exp) is emitted ahead
            # of them and the Act engine sees no pair-boundary bubble.
            assert not pre or max(pre) < len(groups)

            def tail_a():
                # side A closed early (split_tail): evict + norm + projs
                # overlap side B's solo phase
                ytsba = rpool.tile([HD + 1, QB], F32, name="ytsba", tag="ytsba", bufs=2)
                nc.vector.tensor_copy(out=ytsba, in_=yt[:, 0:QB])
                work_q.append(emit_norm(ytsba, r0, r1, qcol, after_a, QB))

            for gi, grp in enumerate(groups):
                took = 0
                for u in (pre or {}).get(gi, ()):
                    u()
                    took += 1
                pv_pending.append(
                    (lambda g, p, ww: lambda: emit_pv(g, p, ww))(*(grp, *emit_s(grp)))
                )
                if split_tail and gi == n_shared // 2 - 1:
                    pv_pending.append(tail_a)
                while len(pv_pending) > 2:
                    pv_pending.popleft()()
                drain_one(gi, took)

            def pair_tail():
                # free the yt PSUM slot; normalization is deferred
                if split_tail:
                    ytsbb = rpool.tile(
                        [HD + 1, QB], F32, name="ytsbb", tag="ytsbb", bufs=2
                    )
                    nc.vector.tensor_copy(out=ytsbb, in_=yt[:, QB : 2 * QB])
                    work_q.append(emit_norm(ytsbb, r0, r1, qcol + QB, after_b, QB))
                else:
                    ytsb = rpool.tile(
                        [HD + 1, 2 * QB], F32, name="ytsb", tag="ytsb", bufs=6
                    )
                    nc.vector.tensor_copy(out=ytsb, in_=yt)
                    work_q.append(
                        emit_norm(ytsb, r0, r1, qcol, tuple(after_a) + tuple(after_b))
                    )

            pv_pending.append(pair_tail)

        # ---- schedule ----
        # Pair processing order: causal-need ascending at the front (pair 7
        # needs the least K/V), and a slot0-only pair LAST so the final
        # norm->proj tail is short and runs on a warm PE.
        order = [7, 6, 5, 3, 2, 1, 0, 4]

        # upfront: exactly what pair 7's first group needs (q-blocks 7,8 +
        # K slice 0 + V blocks 0,1); everything else streams in as
        # deadline-ordered pre-units inside the pair group loops.
        emit_qh(7)
        emit_qh(8)
        emit_k(0)
        emit_v(0)
        emit_v(1)

        def K(ts):
            return lambda: emit_k(ts)

        def Kh(h):
            return lambda: emit_kh(h)

        def V(tb):
            return lambda: emit_v(tb)

        def Q(b):
            return lambda: emit_qh(b)

        # Per-pair pre-unit schedules: K slice s is emitted before the
        # group whose S^T reads it, V block b before the group that emits
        # its P@V (2-deep pending => PV(g) is emitted at group g+2), and
        # the NEXT pairs' q-blocks ride along late in the preceding pair.
        pre_by_pair = {
            7: {1: [K(1), V(2), V(3)], 2: [V(4), V(5)],
                3: [K(2), V(6), V(7)], 4: [Kh(7), V(8), V(9)],
                5: [Kh(6), V(10), V(11), Q(6)],
                6: [Kh(8), V(12), V(13), Q(9)],
                7: [V(14), V(15)], 8: [V(16), V(17)]},
            6: {2: [Kh(9)], 4: [V(18), V(19)], 6: [Q(5)], 7: [Q(10)]},
            5: {2: [K(5)], 4: [V(20), V(21)], 6: [Q(3)], 7: [Q(12)]},
            3: {2: [K(6)], 4: [V(22), V(23)], 5: [V(24), V(25)],
                6: [Q(2)], 7: [Q(13)]},
            2: {2: [K(7)], 4: [V(26), V(27)], 6: [Q(1)], 7: [Q(14)]},
            1: {2: [V(28), V(29)], 6: [Q(0)], 7: [Q(15)]},
            0: {2: [V(30), V(31)], 6: [Q(4)], 7: [Q(11)]},
        }

        for i in order:
            last = i == order[-1]
            full = i in s1_pairs
            pa = [emit_proj(tb, full) for tb in (2 * i, 2 * i + 1)]
            pb = [
                emit_proj(tb, full, act_evict=last)
                for tb in (NKB - 2 - 2 * i, NKB - 1 - 2 * i)
            ]
            if full:
                emit_pair(0, i, pre=pre_by_pair.get(i), split_tail=last)
                emit_pair(1, i, after_a=pa, after_b=pb, split_tail=last)
            else:
                emit_pair(
                    0, i, after_a=pa, after_b=pb,
                    pre=pre_by_pair.get(i), split_tail=last,
                )
        while pv_pending:
            pv_pending.popleft()()
        while work_q:
            work_q.popleft()()

    nc.compile()
    return nc


def _get_ncs():
    if "ncs" not in _CACHE:
        _CACHE["ncs"] = [_build_nc(0), _build_nc(1)]
    return _CACHE["ncs"]


def _core_inputs(x, w_attn, w_proj):
    """Build per-core input dicts (bf16, pre-transposed x, head slices)."""
    import ml_dtypes

    bf16 = ml_dtypes.bfloat16
    xt = np.ascontiguousarray(x.reshape(T, C).T.astype(bf16))
    w_attn = np.asarray(w_attn, dtype=np.float32)
    w_proj = np.asarray(w_proj, dtype=np.float32)
    in_maps = []
    for c in range(N_CORES):
        hF = c
        hH = 8 + (c % 4)
        wa = np.zeros((C, 3, 2, HD), dtype=np.float32)
        wp = np.zeros((2 * HD, C), dtype=np.float32)
        for s, h in enumerate((hF, hH)):
            for p in range(3):
                wa[:, p, s, :] = w_attn[:, p * C + h * HD : p * C + (h + 1) * HD]
            wp[s * HD : (s + 1) * HD, :] = w_proj[h * HD : (h + 1) * HD, :]
        in_maps.append(
            {
                "xt": xt,
                "wa": np.ascontiguousarray(wa.reshape(C, 3 * 2 * HD)).astype(bf16),
                "wp": wp.astype(bf16),
            }
        )
    return in_maps


def _make_sharded(nc, devices):
    """Build one 4-core shard_map'd PJRT executable for a program variant."""
    import jax
    import concourse.mybir as mybir
    from concourse import bass2jax
    from jax.experimental.shard_map import shard_map
    from jax.sharding import Mesh, PartitionSpec

    in_names, out_names, out_avals, zero_outs = [], [], [], []
    for alloc in nc.m.functions[0].allocations:
        if not isinstance(alloc, mybir.MemoryLocationSet):
            continue
        name = alloc.memorylocations[0].name
        if alloc.kind == "ExternalInput":
            if nc.partition_id_tensor and name == nc.partition_id_tensor.name:
                continue
            in_names.append(name)
        elif alloc.kind == "ExternalOutput":
            shape = tuple(alloc.tensor_shape)
            dtype = mybir.dt.np(alloc.dtype)
            out_names.append(name)
            out_avals.append(jax.core.ShapedArray(shape, dtype))
            zero_outs.append(np.zeros(shape, dtype))
    n_params = len(in_names)
    all_in_names = in_names + out_names
    if nc.partition_id_tensor:
        all_in_names = all_in_names + [nc.partition_id_tensor.name]

    def _body(*args):
        operands = list(args)
        if nc.partition_id_tensor:
            operands.append(bass2jax.partition_id_tensor())
        outs = bass2jax._bass_exec_p.bind(
            *operands,
            out_avals=tuple(out_avals),
            in_names=tuple(all_in_names),
            out_names=tuple(out_names),
            lowering_input_output_aliases=(),
            sim_require_finite=True,
            sim_require_nnan=True,
            nc=nc,
        )
        return tuple(outs)

    mesh = Mesh(np.asarray(devices), ("core",))
    n_out = len(out_names)
    donate = tuple(range(n_params, n_params + n_out))
    sharded = jax.jit(
        shard_map(
            _body,
            mesh=mesh,
            in_specs=(PartitionSpec("core"),) * (n_params + n_out),
            out_specs=(PartitionSpec("core"),) * n_out,
            check_rep=False,
        ),
        donate_argnums=donate,
        keep_unused=True,
    )
    return sharded, in_names, out_names, out_avals, zero_outs


def _get_runner():
    if "runner" in _CACHE:
        return _CACHE["runner"]
    import jax
    from concourse import bass2jax

    ncs = _get_ncs()
    bass2jax.install_neuronx_cc_hook()
    devices = jax.devices()[:N_CORES]
    execs = [
        _make_sharded(ncs[0], devices[0:4]),
        _make_sharded(ncs[1], devices[4:8]),
    ]

    def run(in_maps):
        results = [None] * N_CORES
        pending = []
        for v, (sharded, in_names, out_names, out_avals, zero_outs) in enumerate(
            execs
        ):
            cores = range(4 * v, 4 * v + 4)
            concat_in = [
                np.concatenate([in_maps[c][name] for c in cores], axis=0)
                for name in in_names
            ]
            concat_zeros = [
                np.zeros((4 * z.shape[0], *z.shape[1:]), z.dtype) for z in zero_outs
            ]
            out_arrs = sharded(*concat_in, *concat_zeros)
            pending.append((v, out_names, out_avals, out_arrs))
        for v, out_names, out_avals, out_arrs in pending:
            for i, name in enumerate(out_names):
                arr = np.asarray(out_arrs[i]).reshape(4, *out_avals[i].shape)
                for j in range(4):
                    c = 4 * v + j
                    if results[c] is None:
                        results[c] = {}
                    results[c][name] = arr[j]
        return results

    _CACHE["runner"] = run
    return run


def kernel(x, w_attn, w_proj):
    run = _get_runner()
    in_maps = _core_inputs(np.asarray(x), np.asarray(w_attn), np.asarray(w_proj))
    results = run(in_maps)
    out = np.zeros((T, C), dtype=np.float32)
    for c in range(N_CORES):
        out += results[c]["out"].astype(np.float32)
    return out.reshape(1, T, C)

